# revision 22
# baseline (speedup 1.0000x reference)
"""Trainium2 Bass kernel for nn_BasisJastrow.

Math (per batch element b):
    J_b = (1/P) * sum_{i<j} chi_j^T C chi_i ,   P = N(N-1)/2, C = coeff.reshape(Nb, Nb)

Device decomposition (per core; data-parallel over the batch axis).
The 48 per-core batches are split into two halves that live on disjoint SBUF
partition ranges (A = batches 0..23 on partitions 0:64, B = batches 24..47 on
partitions 64:128) so DMA uses all 16 SBUF ports and every PE matmul runs as a
CONCURRENT pair on disjoint systolic-array quadrants:
  A: tile_position (0, 0)    rows 0:63   x cols 0:63    -> PSUM partitions 0:64
  B: tile_position (64, 64)  rows 64:127 x cols 64:127  -> PSUM partitions 64:128

  layout  Xl[n + 64*half, (b,u)]   n=64 on partitions, 24 batches * 32 per half
  phase 1 S  = Lt.T @ Xl           exclusive prefix sums over particles (PE)
  phase 2 Q_p = Xp.T @ Sp          2-batch cross-Gram [64,64] per half; diag
                                   32x32 blocks are G_b, off-diag is garbage
  phase 3 r_p[q] = sum_f Q_p[q,f] * CD2[q,f]   (DVE mul + segmented reduce;
                                   CD2 = blockdiag(C,C)/P masks off-diag)
  phase 4 J = id4.T @ R            partition-block reduction of r columns (PE)

Raw Bass (explicit engine blocks + semaphores): the walrus build in this
container rejects any instruction carrying more than one sync wait, which
rules out Tile's generated sem placement; raw Bass emits waits standalone.
"""

import sys

for _p in ("/opt/trn_rl_repo",):
    if _p not in sys.path:
        sys.path.insert(0, _p)

import numpy as np

import concourse.bass as bass
from concourse import mybir
from concourse.bass_utils import run_bass_kernel_spmd

B, N, Nb = 384, 64, 32
NCORES = 8
BS = B // NCORES            # 48 batches per core
HB = BS // 2                # 24 batches per half
NP = HB // 2                # 12 concurrent gram pairs
NPAIR = N * (N - 1) // 2    # 2016
F32 = mybir.dt.float32

USE_BF16 = True
MM_DT = mybir.dt.bfloat16 if USE_BF16 else F32

FREE = HB * Nb              # 768 free columns per half
CHUNK = 384
NCHUNK = FREE // CHUNK      # 2
NQB = 4                     # gram psum banks
PPB = 3                     # gram pairs per bank

# const pack layout: cols [64:68] id4, [68:132] CD2 (Lt is its own tensor)
CP_COLS = 132


def build_nc() -> bass.Bass:
    nc = bass.Bass()

    x_d = nc.dram_tensor("x", [128, FREE], MM_DT, kind="ExternalInput")
    cp_d = nc.dram_tensor("cp", [128, CP_COLS], F32, kind="ExternalInput")
    lt_d = nc.dram_tensor("lt", [128, N], MM_DT, kind="ExternalInput")
    j_d = nc.dram_tensor("j", [4, NP], F32, kind="ExternalOutput")

    from contextlib import ExitStack

    with ExitStack() as ctx:
        x_sb = ctx.enter_context(nc.sbuf_tensor("x_sb", [128, FREE], MM_DT))
        s_sb = ctx.enter_context(nc.sbuf_tensor("s_sb", [128, FREE], MM_DT))
        cp_sb = ctx.enter_context(nc.sbuf_tensor("cp_sb", [128, CP_COLS], F32))
        lt_sb = ctx.enter_context(nc.sbuf_tensor("lt_sb", [128, N], MM_DT))
        e_sb = ctx.enter_context(nc.sbuf_tensor("e_sb", [128, NQB, PPB * 64], F32))
        r_sb = ctx.enter_context(nc.sbuf_tensor("r_sb", [128, NP], F32))
        j_sb = ctx.enter_context(nc.sbuf_tensor("j_sb", [4, NP], F32))
        s_ps = [
            ctx.enter_context(nc.psum_tensor(f"s_ps{c}", [128, CHUNK], F32))
            for c in range(NCHUNK)
        ]
        q_ps = [
            ctx.enter_context(nc.psum_tensor(f"q_ps{k}", [128, PPB * 64], F32))
            for k in range(NQB)
        ]
        jw_ps = ctx.enter_context(nc.psum_tensor("jw_ps", [4, 64], F32))
        dma_c = ctx.enter_context(nc.semaphore("dma_c"))
        dma_l = ctx.enter_context(nc.semaphore("dma_l"))
        dma_x = [
            ctx.enter_context(nc.semaphore(f"dma_x{c}")) for c in range(NCHUNK)
        ]
        dma_o = ctx.enter_context(nc.semaphore("dma_o"))
        pe_s = ctx.enter_context(nc.semaphore("pe_s"))
        pe_q = ctx.enter_context(nc.semaphore("pe_q"))
        pe_j = ctx.enter_context(nc.semaphore("pe_j"))
        dve_c = ctx.enter_context(nc.semaphore("dve_c"))
        dve_m = ctx.enter_context(nc.semaphore("dve_m"))
        dve_t = ctx.enter_context(nc.semaphore("dve_t"))
        act_j = ctx.enter_context(nc.semaphore("act_j"))
        block = ctx.enter_context(nc.Block())
        id4 = cp_sb[:, 64:68]
        cd2 = cp_sb[:, 68:132]
        PPC = NP // NCHUNK  # gram pairs per x-chunk

        @block.sync
        def _(sync):
            sync.dma_start(out=lt_sb[:], in_=lt_d[:]).then_inc(dma_l, 16)
            for c in range(NCHUNK):
                cs = slice(c * CHUNK, (c + 1) * CHUNK)
                sync.dma_start(out=x_sb[:, cs], in_=x_d[:, cs]).then_inc(
                    dma_x[c], 16
                )
            sync.wait_ge(act_j, 1)
            sync.dma_start(out=j_d[:], in_=j_sb[:]).then_inc(dma_o, 16)
            sync.wait_ge(dma_o, 16)

        @block.scalar
        def _(scalar):
            scalar.dma_start(out=cp_sb[:], in_=cp_d[:]).then_inc(dma_c, 16)

        @block.tensor
        def _(tensor):
            tensor.wait_ge(dma_l, 16)
            # phase 1: exclusive prefix sums, concurrent halves
            for c in range(NCHUNK):
                cs = slice(c * CHUNK, (c + 1) * CHUNK)
                tensor.wait_ge(dma_x[c], 16)
                tensor.matmul(
                    s_ps[c][0:64, :],
                    lt_sb[0:64, :],
                    x_sb[0:64, cs],
                    start=True,
                    stop=True,
                    tile_position=(0, 0),
                )
                tensor.matmul(
                    s_ps[c][64:128, :],
                    lt_sb[64:128, :],
                    x_sb[64:128, cs],
                    start=True,
                    stop=True,
                    tile_position=(64, 64),
                ).then_inc(pe_s, 1)
            # phase 2: 2-batch cross-Grams, concurrent halves
            for p in range(NP):
                ps_ = slice(p * 64, (p + 1) * 64)
                if p % PPC == 0:
                    tensor.wait_ge(dve_c, p // PPC + 1)
                q = q_ps[p // PPB][:, (p % PPB) * 64 : (p % PPB + 1) * 64]
                tensor.matmul(
                    q[0:64, :],
                    x_sb[0:64, ps_],
                    s_sb[0:64, ps_],
                    start=True,
                    stop=True,
                    tile_position=(0, 0),
                )
                tensor.matmul(
                    q[64:128, :],
                    x_sb[64:128, ps_],
                    s_sb[64:128, ps_],
                    start=True,
                    stop=True,
                    tile_position=(64, 64),
                ).then_inc(pe_q, 1)
            # phase 4: partition-block reduction of r columns
            for k in range(NQB):
                tensor.wait_ge(dve_t, k + 1)
                ks = slice(k * PPB, (k + 1) * PPB)
                tensor.matmul(
                    jw_ps[0:4, k * PPB : (k + 1) * PPB],
                    id4,
                    r_sb[:, ks],
                    start=True,
                    stop=True,
                ).then_inc(pe_j, 1)

        @block.vector
        def _(vector):
            for c in range(NCHUNK):
                cs = slice(c * CHUNK, (c + 1) * CHUNK)
                vector.wait_ge(pe_s, c + 1)
                vector.tensor_copy(s_sb[:, cs], s_ps[c][:]).then_inc(dve_c, 1)

            cd4 = bass.AP(
                tensor=cd2.tensor,
                offset=cd2.offset,
                ap=[list(cd2.ap[0]), [0, PPB], list(cd2.ap[1])],
            )

            def mul(b):
                if b == 0:
                    vector.wait_ge(dma_c, 16)
                vector.wait_ge(pe_q, PPB * (b + 1))
                q4 = q_ps[b][:].rearrange("p (r f) -> p r f", r=PPB)
                e4 = e_sb[:, b, :].rearrange("p (r f) -> p r f", r=PPB)
                vector.tensor_tensor(
                    out=e4, in0=q4, in1=cd4, op=mybir.AluOpType.mult
                ).then_inc(dve_m, 1)

            def red(b):
                vector.wait_ge(dve_m, b + 1)
                e4 = e_sb[:, b, :].rearrange("p (r f) -> p r f", r=PPB)
                vector.tensor_reduce(
                    out=r_sb[:, PPB * b : PPB * (b + 1)],
                    in_=e4,
                    axis=mybir.AxisListType.X,
                    op=mybir.AluOpType.add,
                ).then_inc(dve_t, 1)

            mul(0)
            mul(1)
            red(0)
            mul(2)
            red(1)
            mul(3)
            red(2)
            red(3)
            vector.wait_ge(pe_j, NQB)
            vector.tensor_copy(j_sb[:], jw_ps[0:4, 0:NP]).then_inc(act_j, 1)

    return nc


def _np_mm_dtype():
    if USE_BF16:
        import ml_dtypes

        return ml_dtypes.bfloat16
    return np.float32


def make_consts(jastrow_coeff: np.ndarray):
    C = np.asarray(jastrow_coeff, dtype=np.float32).reshape(Nb, Nb)
    cp = np.zeros((128, CP_COLS), dtype=np.float32)
    bd2 = np.zeros((64, 64), dtype=np.float32)
    for i in range(2):
        bd2[32 * i : 32 * (i + 1), 32 * i : 32 * (i + 1)] = C / NPAIR
    cp[0:64, 68:132] = bd2
    cp[64:128, 68:132] = bd2
    for i in range(4):
        cp[32 * i : 32 * (i + 1), 64 + i] = 1.0
    lt1 = np.triu(np.ones((N, N), dtype=np.float32), k=1)
    lt = np.concatenate([lt1, lt1], axis=0).astype(_np_mm_dtype())
    return cp, lt


def shard_x(basis_single_body: np.ndarray):
    x = np.asarray(basis_single_body, dtype=np.float32)
    xt = np.ascontiguousarray(x.transpose(1, 0, 2))  # [N, B, Nb]
    dt = _np_mm_dtype()
    out = []
    for m in range(NCORES):
        sl = xt[:, m * BS : (m + 1) * BS, :]
        a = sl[:, 0:HB, :].reshape(N, FREE)
        b = sl[:, HB:BS, :].reshape(N, FREE)
        out.append(np.ascontiguousarray(np.concatenate([a, b], axis=0)).astype(dt))
    return out


def unpack_j(j: np.ndarray) -> np.ndarray:
    """j[i, p] -> per-core J[48]: blocks 0,1 = half A batch 2p+i,
    blocks 2,3 = half B batch 24+2p+(i-2)."""
    ja = j[0:2, :].T.ravel()
    jb = j[2:4, :].T.ravel()
    return np.concatenate([ja, jb]).astype(np.float32)


_NC_CACHE: list = []


def kernel(basis_single_body: np.ndarray, jastrow_coeff: np.ndarray) -> np.ndarray:
    if not _NC_CACHE:
        _NC_CACHE.append(build_nc())
    nc = _NC_CACHE[0]

    cp, lt = make_consts(jastrow_coeff)
    shards = shard_x(basis_single_body)
    in_maps = [{"x": s, "cp": cp, "lt": lt} for s in shards]

    res = run_bass_kernel_spmd(nc, in_maps, core_ids=list(range(NCORES)))
    return np.concatenate([unpack_j(np.asarray(r["j"])) for r in res.results])


# revision 23
# speedup vs baseline: 1.0223x; 1.0223x over previous
"""Trainium2 Bass kernel for nn_BasisJastrow.

Math (per batch element b):
    J_b = (1/P) * sum_{i<j} chi_j^T C chi_i ,   P = N(N-1)/2, C = coeff.reshape(Nb, Nb)

Device decomposition (per core; data-parallel over the batch axis).
The 48 per-core batches are split into two halves that live on disjoint SBUF
partition ranges (A = batches 0..23 on partitions 0:64, B = batches 24..47 on
partitions 64:128) so DMA uses all 16 SBUF ports and every PE matmul runs as a
CONCURRENT pair on disjoint systolic-array quadrants:
  A: tile_position (0, 0)    rows 0:63   x cols 0:63    -> PSUM partitions 0:64
  B: tile_position (64, 64)  rows 64:127 x cols 64:127  -> PSUM partitions 64:128

  layout  Xl[n + 64*half, (b,u)]   n=64 on partitions, 24 batches * 32 per half
  phase 1 S  = Lt.T @ Xl           exclusive prefix sums over particles (PE)
  phase 2 Q_p = Xp.T @ Sp          2-batch cross-Gram [64,64] per half; diag
                                   32x32 blocks are G_b, off-diag is garbage
  phase 3 r_p[q] = sum_f Q_p[q,f] * CD2[q,f]   (DVE mul + segmented reduce;
                                   CD2 = blockdiag(C,C)/P masks off-diag)
  phase 4 J = id4.T @ R            partition-block reduction of r columns (PE)

Raw Bass (explicit engine blocks + semaphores): the walrus build in this
container rejects any instruction carrying more than one sync wait, which
rules out Tile's generated sem placement; raw Bass emits waits standalone.
"""

import sys

for _p in ("/opt/trn_rl_repo",):
    if _p not in sys.path:
        sys.path.insert(0, _p)

import numpy as np

import concourse.bass as bass
from concourse import mybir
from concourse.bass_utils import run_bass_kernel_spmd

B, N, Nb = 384, 64, 32
NCORES = 8
BS = B // NCORES            # 48 batches per core
HB = BS // 2                # 24 batches per half
NP = HB // 2                # 12 concurrent gram pairs
NPAIR = N * (N - 1) // 2    # 2016
F32 = mybir.dt.float32

USE_BF16 = True
MM_DT = mybir.dt.bfloat16 if USE_BF16 else F32

FREE = HB * Nb              # 768 free columns per half
CHUNK = 256
NCHUNK = FREE // CHUNK      # 3
NQB = 4                     # gram psum banks
PPB = 3                     # gram pairs per bank

# const pack layout: cols [64:68] id4, [68:132] CD2 (Lt is its own tensor)
CP_COLS = 132


def build_nc() -> bass.Bass:
    nc = bass.Bass()

    x_d = nc.dram_tensor("x", [128, FREE], MM_DT, kind="ExternalInput")
    cp_d = nc.dram_tensor("cp", [128, CP_COLS], F32, kind="ExternalInput")
    lt_d = nc.dram_tensor("lt", [128, N], MM_DT, kind="ExternalInput")
    j_d = nc.dram_tensor("j", [4, NP], F32, kind="ExternalOutput")

    from contextlib import ExitStack

    with ExitStack() as ctx:
        x_sb = ctx.enter_context(nc.sbuf_tensor("x_sb", [128, FREE], MM_DT))
        s_sb = ctx.enter_context(nc.sbuf_tensor("s_sb", [128, FREE], MM_DT))
        cp_sb = ctx.enter_context(nc.sbuf_tensor("cp_sb", [128, CP_COLS], F32))
        lt_sb = ctx.enter_context(nc.sbuf_tensor("lt_sb", [128, N], MM_DT))
        e_sb = ctx.enter_context(nc.sbuf_tensor("e_sb", [128, NQB, PPB * 64], F32))
        r_sb = ctx.enter_context(nc.sbuf_tensor("r_sb", [128, NP], F32))
        j_sb = ctx.enter_context(nc.sbuf_tensor("j_sb", [4, NP], F32))
        s_ps = [
            ctx.enter_context(nc.psum_tensor(f"s_ps{c}", [128, CHUNK], F32))
            for c in range(NCHUNK)
        ]
        q_ps = [
            ctx.enter_context(nc.psum_tensor(f"q_ps{k}", [128, PPB * 64], F32))
            for k in range(NQB)
        ]
        jw_ps = ctx.enter_context(nc.psum_tensor("jw_ps", [4, 64], F32))
        dma_c = ctx.enter_context(nc.semaphore("dma_c"))
        dma_l = ctx.enter_context(nc.semaphore("dma_l"))
        dma_x = [
            ctx.enter_context(nc.semaphore(f"dma_x{c}")) for c in range(NCHUNK)
        ]
        dma_o = ctx.enter_context(nc.semaphore("dma_o"))
        pe_s = ctx.enter_context(nc.semaphore("pe_s"))
        pe_q = ctx.enter_context(nc.semaphore("pe_q"))
        pe_j = ctx.enter_context(nc.semaphore("pe_j"))
        dve_c = ctx.enter_context(nc.semaphore("dve_c"))
        dve_m = ctx.enter_context(nc.semaphore("dve_m"))
        dve_t = ctx.enter_context(nc.semaphore("dve_t"))
        act_j = ctx.enter_context(nc.semaphore("act_j"))
        block = ctx.enter_context(nc.Block())
        id4 = cp_sb[:, 64:68]
        cd2 = cp_sb[:, 68:132]
        PPC = NP // NCHUNK  # gram pairs per x-chunk

        @block.sync
        def _(sync):
            for c in range(NCHUNK):
                cs = slice(c * CHUNK, (c + 1) * CHUNK)
                sync.dma_start(out=x_sb[:, cs], in_=x_d[:, cs]).then_inc(
                    dma_x[c], 16
                )
            sync.wait_ge(act_j, 1)
            sync.dma_start(out=j_d[:], in_=j_sb[:]).then_inc(dma_o, 16)
            sync.wait_ge(dma_o, 16)

        @block.scalar
        def _(scalar):
            scalar.dma_start(out=lt_sb[:], in_=lt_d[:]).then_inc(dma_l, 16)
            scalar.dma_start(out=cp_sb[:], in_=cp_d[:]).then_inc(dma_c, 16)

        @block.tensor
        def _(tensor):
            tensor.wait_ge(dma_l, 16)
            # phase 1: exclusive prefix sums, concurrent halves
            for c in range(NCHUNK):
                cs = slice(c * CHUNK, (c + 1) * CHUNK)
                tensor.wait_ge(dma_x[c], 16)
                tensor.matmul(
                    s_ps[c][0:64, :],
                    lt_sb[0:64, :],
                    x_sb[0:64, cs],
                    start=True,
                    stop=True,
                    tile_position=(0, 0),
                )
                tensor.matmul(
                    s_ps[c][64:128, :],
                    lt_sb[64:128, :],
                    x_sb[64:128, cs],
                    start=True,
                    stop=True,
                    tile_position=(64, 64),
                ).then_inc(pe_s, 1)
            # phase 2: 2-batch cross-Grams, concurrent halves
            for p in range(NP):
                ps_ = slice(p * 64, (p + 1) * 64)
                if p % PPC == 0:
                    tensor.wait_ge(dve_c, p // PPC + 1)
                q = q_ps[p // PPB][:, (p % PPB) * 64 : (p % PPB + 1) * 64]
                tensor.matmul(
                    q[0:64, :],
                    x_sb[0:64, ps_],
                    s_sb[0:64, ps_],
                    start=True,
                    stop=True,
                    tile_position=(0, 0),
                )
                tensor.matmul(
                    q[64:128, :],
                    x_sb[64:128, ps_],
                    s_sb[64:128, ps_],
                    start=True,
                    stop=True,
                    tile_position=(64, 64),
                ).then_inc(pe_q, 1)
            # phase 4: partition-block reduction of r columns
            for k in range(NQB):
                tensor.wait_ge(dve_t, k + 1)
                ks = slice(k * PPB, (k + 1) * PPB)
                tensor.matmul(
                    jw_ps[0:4, k * PPB : (k + 1) * PPB],
                    id4,
                    r_sb[:, ks],
                    start=True,
                    stop=True,
                ).then_inc(pe_j, 1)

        @block.vector
        def _(vector):
            for c in range(NCHUNK):
                cs = slice(c * CHUNK, (c + 1) * CHUNK)
                vector.wait_ge(pe_s, c + 1)
                vector.tensor_copy(s_sb[:, cs], s_ps[c][:]).then_inc(dve_c, 1)

            cd4 = bass.AP(
                tensor=cd2.tensor,
                offset=cd2.offset,
                ap=[list(cd2.ap[0]), [0, PPB], list(cd2.ap[1])],
            )

            def mul(b):
                if b == 0:
                    vector.wait_ge(dma_c, 16)
                vector.wait_ge(pe_q, PPB * (b + 1))
                q4 = q_ps[b][:].rearrange("p (r f) -> p r f", r=PPB)
                e4 = e_sb[:, b, :].rearrange("p (r f) -> p r f", r=PPB)
                vector.tensor_tensor(
                    out=e4, in0=q4, in1=cd4, op=mybir.AluOpType.mult
                ).then_inc(dve_m, 1)

            def red(b):
                vector.wait_ge(dve_m, b + 1)
                e4 = e_sb[:, b, :].rearrange("p (r f) -> p r f", r=PPB)
                vector.tensor_reduce(
                    out=r_sb[:, PPB * b : PPB * (b + 1)],
                    in_=e4,
                    axis=mybir.AxisListType.X,
                    op=mybir.AluOpType.add,
                ).then_inc(dve_t, 1)

            mul(0)
            mul(1)
            red(0)
            mul(2)
            red(1)
            mul(3)
            red(2)
            red(3)
            vector.wait_ge(pe_j, NQB)
            vector.tensor_copy(j_sb[:], jw_ps[0:4, 0:NP]).then_inc(act_j, 1)

    return nc


def _np_mm_dtype():
    if USE_BF16:
        import ml_dtypes

        return ml_dtypes.bfloat16
    return np.float32


def make_consts(jastrow_coeff: np.ndarray):
    C = np.asarray(jastrow_coeff, dtype=np.float32).reshape(Nb, Nb)
    cp = np.zeros((128, CP_COLS), dtype=np.float32)
    bd2 = np.zeros((64, 64), dtype=np.float32)
    for i in range(2):
        bd2[32 * i : 32 * (i + 1), 32 * i : 32 * (i + 1)] = C / NPAIR
    cp[0:64, 68:132] = bd2
    cp[64:128, 68:132] = bd2
    for i in range(4):
        cp[32 * i : 32 * (i + 1), 64 + i] = 1.0
    lt1 = np.triu(np.ones((N, N), dtype=np.float32), k=1)
    lt = np.concatenate([lt1, lt1], axis=0).astype(_np_mm_dtype())
    return cp, lt


def shard_x(basis_single_body: np.ndarray):
    x = np.asarray(basis_single_body, dtype=np.float32)
    xt = np.ascontiguousarray(x.transpose(1, 0, 2))  # [N, B, Nb]
    dt = _np_mm_dtype()
    out = []
    for m in range(NCORES):
        sl = xt[:, m * BS : (m + 1) * BS, :]
        a = sl[:, 0:HB, :].reshape(N, FREE)
        b = sl[:, HB:BS, :].reshape(N, FREE)
        out.append(np.ascontiguousarray(np.concatenate([a, b], axis=0)).astype(dt))
    return out


def unpack_j(j: np.ndarray) -> np.ndarray:
    """j[i, p] -> per-core J[48]: blocks 0,1 = half A batch 2p+i,
    blocks 2,3 = half B batch 24+2p+(i-2)."""
    ja = j[0:2, :].T.ravel()
    jb = j[2:4, :].T.ravel()
    return np.concatenate([ja, jb]).astype(np.float32)


_NC_CACHE: list = []


def kernel(basis_single_body: np.ndarray, jastrow_coeff: np.ndarray) -> np.ndarray:
    if not _NC_CACHE:
        _NC_CACHE.append(build_nc())
    nc = _NC_CACHE[0]

    cp, lt = make_consts(jastrow_coeff)
    shards = shard_x(basis_single_body)
    in_maps = [{"x": s, "cp": cp, "lt": lt} for s in shards]

    res = run_bass_kernel_spmd(nc, in_maps, core_ids=list(range(NCORES)))
    return np.concatenate([unpack_j(np.asarray(r["j"])) for r in res.results])
